# revision 8
# baseline (speedup 1.0000x reference)
"""NTK-ViT self-attention (softmax attention + linear-attention correction)
for Trainium2, data-parallel over batch across 8 NeuronCores.

Math (per batch b, head h):
    q = hidden @ Wq.T + bq ; k = .. ; v = ..
    A'  = exp(q k^T / sqrt(d))          (no max-subtract; cancels in ratio)
    phi_q = elu(q / d^0.25) + 1 = exp(min(x,0)) + relu(x)
    ctx = (A' v + phi_q @ phi_kv) / (rowsum(A') + phi_q @ |phi_k|)

Design (per core: 2 batches, 12 heads = 6 head pairs):
  - fp16 datapath end to end (11x more accurate than bf16 here: every
    intermediate fits fp16 range; A' <= exp(max logit) ~ 250 << 65504).
    Matmul rate is identical to bf16; HW rel err ~9e-4 vs 2e-2 budget.
  - hidden is pre-cast to fp16 on host and transposed by the DMA xbar
    (dma_start(transpose=True)) directly into SBUF hT [128, itile, S]
    (row i = itile*128 + p, matching the W^T layout) - no PE transposes,
    no DVE copies, and the preamble's PE time drops ~19 us/rep.
  - all of W^T (fp16, 27.6 KB/partition) is resident in SBUF, loaded once
    at boot; projections never touch DRAM again. This also keeps the SP
    DMA queue free: a blocked DMA at a queue head serializes every later
    DMA on real HW (strict FIFO), which is what serialized reps in the
    old kernel.
  - output flush DMAs go through the Pool-engine SWDGE queue so the SP
    queue only ever carries boot loads + hT transposes (no cross-rep
    FIFO stalls). The output DRAM tensor is partition-major [b, p, qt, c]
    (the SBUF buffer's natural layout) so flushes are contiguous fused
    2-qt DMAs; the host transposes rows back after the gather.
  - scores computed transposed: S^T[t, q]; the two heads of a pair run
    as row-tiled matmuls (tile_position (0,0)/(64,0), K=64) which the PE
    can execute concurrently.
  - exp on ACT (scale=1/8 fused) from PSUM (172-cycle access) writing
    fp16 A'^T tiles; ACT is ~212 us/rep busy and is the second-longest
    engine after PE (~230 us/rep), so the emission order keeps the exp
    stream fed: per head pair, Q/K projections and the V projection are
    interleaved between att_qk/att_av blocks; batch 1's preamble rides
    inside batch 0's attention, and the next rep's preamble fills the
    ACT-bound tail (sim: 253 us single-shot, ~224 us marginal rep cost,
    228.0 us on the r9-slope harness metric, vs 348/353 us baseline). AV runs under a priority boost so attention
    output never lags the exp stream (AT buffer recycling).
  - A'v: out[q_tile, 65] PSUM accumulation with lhsT = A'^T tiles (fp16)
    and rhs = V_aug = [V | ones] (fp16); the ones column yields
    rowsum(A') free. The phi-correction matmul (K=64, row-tiled)
    accumulates into the same bank. DVE reciprocal of col 64 scales
    cols 0..63 into the fp16 out_sb, flushed per-qt on last head pairs.
  - 10 zero matmuls at boot ramp the PE HAM clock gate (1.2 -> 2.4 GHz)
    while the first DMAs are in flight; a dummy exp preloads the ACT
    table set (~2.7 us) off the critical path.
"""

import contextlib

import numpy as np

import concourse.bass as bass
import concourse.mybir as mybir
import concourse.tile as tile
import bass_rust
from concourse.bass_utils import run_bass_kernel_spmd

F32 = mybir.dt.float32
F16 = mybir.dt.float16
F8 = mybir.dt.float16  # BISECT: fp8 off          # e4m3: A' <= ~250 < 448 max
AF = mybir.ActivationFunctionType
ALU = mybir.AluOpType

B, S, HID = 16, 1024, 768
H, DH = 12, 64
NCORES = 8
BLOC = B // NCORES          # batches per core
SLOC = BLOC * S             # 2048 rows of hidden per core
HP = H // 2                 # head pairs
INV_SQRT_D = 1.0 / np.sqrt(DH)          # 0.125
INV_QD = 1.0 / float(DH) ** 0.25        # 1/2.8284


def _split_multi_waits(nc):
    """This walrus build rejects instructions carrying more than one sync
    wait. Hoist extra waits onto standalone EventSemaphore nops emitted
    immediately before the instruction on the same engine (identical
    blocking semantics: the engine stalls on each wait in turn)."""
    ctr = 0
    for fn in nc.m.functions:
        for bb in fn.blocks:
            out = []
            changed = False
            for inst in bb.instructions:
                si = inst.sync_info
                if si is not None and len(si.on_wait) > 1:
                    waits = list(si.on_wait)
                    for w in waits[:-1]:
                        ctr += 1
                        nop = mybir.InstEventSemaphore(
                            name=f"I-waitsplit-{ctr}",
                            engine=inst.engine,
                            ins=[], outs=[],
                            sync_info=bass_rust.SyncInfo(
                                on_wait=[w], on_update=[]),
                        )
                        out.append(nop)
                    inst.sync_info = bass_rust.SyncInfo(
                        on_wait=[waits[-1]], on_update=list(si.on_update))
                    changed = True
                out.append(inst)
            if changed:
                bb.instructions = out


def build_nc(split_waits=True, reps=1):
    nc = bass.Bass()
    hid = nc.declare_dram_parameter("hidden", [SLOC, HID], F16, isOutput=False)
    # W^T pre-transposed on host into the SBUF layout [p_i, i_tile, o]
    wt_in = {
        w: nc.declare_dram_parameter(f"{w}t", [128, 6, HID], F16,
                                     isOutput=False)
        for w in ("wq", "wk", "wv")
    }
    bq_p = nc.declare_dram_parameter("bq_p", [128, 6], F32, isOutput=False)
    bk_p = nc.declare_dram_parameter("bk_p", [128, 6], F32, isOutput=False)
    bv_b = nc.declare_dram_parameter("bv_b", [128, HID], F32, isOutput=False)
    # [128 (=2x d), H, DH+1]: [phi_kv | |phi_k|], replicated on both 64-halves
    pkv = nc.declare_dram_parameter("phikv_aug", [128, H, DH + 1], F32,
                                    isOutput=False)
    # p-major output: [b, p, qt, c]; host transposes back to row order
    outp = nc.declare_dram_parameter("out", [BLOC, 128, 8, HID], F16,
                                     isOutput=True)

    with tile.TileContext(nc) as tc:
        with contextlib.ExitStack() as ctx:
            cpool = ctx.enter_context(tc.tile_pool(name="const", bufs=1))
            big = ctx.enter_context(tc.tile_pool(name="big", bufs=1))
            htp = ctx.enter_context(tc.tile_pool(name="htp", bufs=2))
            att = ctx.enter_context(tc.tile_pool(name="att", bufs=4))
            prp = ctx.enter_context(tc.tile_pool(name="prp", bufs=6))
            vap = ctx.enter_context(tc.tile_pool(name="vap", bufs=2))
            tmp = ctx.enter_context(tc.tile_pool(name="tmp", bufs=2))
            eps_p = ctx.enter_context(tc.tile_pool(name="eps", bufs=6))
            pmix = ctx.enter_context(tc.tile_pool(name="pmix", bufs=2,
                                                  space="PSUM"))
            pqk = ctx.enter_context(tc.tile_pool(name="pqk", bufs=2,
                                                 space="PSUM"))
            pav = ctx.enter_context(tc.tile_pool(name="pav", bufs=2,
                                                 space="PSUM"))

            pool_eng = nc.gpsimd

            # ---- PE warmup: dummy matmuls ramp the HAM clock gate while
            # the first DMAs are in flight ----
            def emit_warmup():
                wz = tmp.tile([128, 512], F16, tag="warm", name="warmz")
                nc.vector.memset(wz[:], 0.0)
                # preload the exp table set while DMAs stream in
                wze = tmp.tile([128, 8], F16, tag="warme", name="warme")
                nc.scalar.activation(wze[:], wz[:, 0:8], AF.Exp)
                for i in range(20):
                    pw = pmix.tile([128, 512], F32, tag="mix", name="pw")
                    nc.tensor.matmul(pw[:], lhsT=wz[:, 0:128], rhs=wz[:],
                                     start=True, stop=True)

            # ---- resident weights: all of W^T (bf16) lives in SBUF ----
            def alloc_weights():
                return {wkey: cpool.tile([128, 6, HID], F16,
                                         name=f"wres_{wkey}")
                        for wkey in ("wq", "wk", "wv")}

            def emit_weight_cols(wkey, c0, c1):
                nc.sync.dma_start(
                    out=wres[wkey][:, :, c0:c1],
                    in_=wt_in[wkey][:, :, c0:c1])

            def emit_consts_biases():
                bqt = cpool.tile([128, 6], F32)
                nc.sync.dma_start(out=bqt[:], in_=bq_p[:])
                bkt = cpool.tile([128, 6], F32)
                nc.sync.dma_start(out=bkt[:], in_=bk_p[:])
                return (bqt, bkt)

            def emit_consts_rest():
                pkv_f = tmp.tile([128, H, DH + 1], F32, tag="boot",
                                 name="pkv_f")
                nc.sync.dma_start(out=pkv_f[:], in_=pkv[:])
                pkv_b = cpool.tile([128, H, DH + 1], F16)
                nc.vector.tensor_copy(out=pkv_b[:], in_=pkv_f[:])
                bvt = cpool.tile([128, HID], F32)
                nc.sync.dma_start(out=bvt[:], in_=bv_b[:])
                return (pkv_b, bvt)

            def emit_hT(b, hT, sc=None):
                # hidden [S, HID] bf16 --xbar--> hT [128, it, S]
                # (row i = it*128 + p, matching the W^T SBUF layout)
                # two half-DMAs so the first projection chunk (qc=0, s<512)
                # starts as soon as the first half lands
                scs = range(2) if sc is None else [sc]
                for sc_ in scs:
                    nc.sync.dma_start(
                        out=hT[:, :, sc_ * 512:(sc_ + 1) * 512],
                        in_=hid[b * S + sc_ * 512: b * S + (sc_ + 1) * 512, :],
                        transpose=True)

            def emit_proj_hp(b, stt, wkey, dsts, bias, tg, with_phi, hp):
                hT = stt["hT"]
                o0 = hp * 128
                dst = prp.tile([128, S], F16, tag=tg, name=f"{tg}_{b}_{hp}")
                dsts[hp] = dst
                for qc in range(2):
                    ps = pmix.tile([128, 512], F32, tag="mix", name="ps")
                    for it in range(6):
                        nc.tensor.matmul(
                            ps[:],
                            lhsT=wres[wkey][:, it, o0:o0 + 128],
                            rhs=hT[:, it, qc * 512:(qc + 1) * 512],
                            start=(it == 0), stop=(it == 5))
                    nc.vector.tensor_scalar(
                        out=dst[:, qc * 512:(qc + 1) * 512],
                        in0=ps[:], scalar1=bias[:, hp:hp + 1],
                        scalar2=None, op0=ALU.add)
                if with_phi:
                    phiQ = prp.tile([128, S], F16, tag="phiQp",
                                    name=f"phiQ_{b}_{hp}")
                    stt["phiQs"][hp] = phiQ
                    t1 = tmp.tile([128, S], F16, tag="phi_min")
                    nc.vector.tensor_scalar_min(t1[:], dst[:], 0.0)
                    nc.scalar.activation(phiQ[:], t1[:], AF.Exp,
                                         scale=INV_QD)
                    t1b = tmp.tile([128, S], F16, tag="phi_min")
                    nc.vector.tensor_scalar(
                        out=t1b[:], in0=dst[:],
                        scalar1=0.0, scalar2=INV_QD, op0=ALU.max,
                        op1=ALU.mult)
                    nc.vector.tensor_tensor(
                        out=phiQ[:], in0=phiQ[:], in1=t1b[:], op=ALU.add)

            def emit_proj(b, stt, wkey, dsts, bias, tg, with_phi, half):
                for hp in range(half * 3, half * 3 + 3):
                    emit_proj_hp(b, stt, wkey, dsts, bias, tg, with_phi, hp)

            def emit_v(b, stt, half):
                bvt = consts[3]  # (bqt, bkt, pkv_b, bvt)
                hT = stt["hT"]
                wf_v = wres["wv"][:, :, half * 384:(half + 1) * 384]
                if half == 0:
                    vaug = vap.tile([128, 8, H, DH + 1], F8, tag="vaug",
                                    name=f"vaug_{b}")
                    stt["vaug"] = vaug
                    nc.any.memset(vaug[:, :, :, DH:DH + 1], 1.0)
                vaug = stt["vaug"]
                for st in range(8):
                    ps = pmix.tile([128, 512], F32, tag="mix", name="ps")
                    for it in range(6):
                        nc.tensor.matmul(
                            ps[:, :384],
                            lhsT=hT[:, it, st * 128:(st + 1) * 128],
                            rhs=wf_v[:, it, 0:384],
                            start=(it == 0), stop=(it == 5))
                    nc.vector.tensor_tensor(
                        out=vaug[:, st, half * 6:(half + 1) * 6, 0:DH],
                        in0=ps[:, :384].rearrange("p (h d) -> p h d", d=DH),
                        in1=bvt[:, half * 384:(half + 1) * 384].rearrange(
                            "p (h d) -> p h d", d=DH),
                        op=ALU.add)

            def new_state(b):
                return dict(QTs=[None] * HP, KTs=[None] * HP,
                            phiQs=[None] * HP, hT=None, vaug=None,
                            out_sb=big.tile([128, 8, HID], F16, tag="out_sb",
                                            name=f"osb_{b}"))

            def att_qk(b, stt, hp):
                # the two heads of a pair are row-tiled (T0 rows 0-63 /
                # T8 rows 64-127); emitting the matmuls h2-interleaved
                # (A-qc0, B-qc0, A-qc1, B-qc1) lets the PE stream both
                # tiles CONCURRENTLY (dstart ~4ns) instead of serially.
                QTs, KTs = stt["QTs"], stt["KTs"]
                ATh = [att.tile([128, 8, S], F8, tag="AT",
                                name=f"AT_{b}_{hp}_{h2}")
                       for h2 in range(2)]
                with tc.high_priority():
                    for t in range(8):
                        ps2 = [pqk.tile([128, 1024], F32, tag="qk",
                                        name="psqk")
                               for _ in range(2)]
                        for qc in range(2):
                            for h2 in range(2):
                                lo, hi = h2 * 64, (h2 + 1) * 64
                                nc.tensor.matmul(
                                    ps2[h2][:, qc * 512:(qc + 1) * 512],
                                    lhsT=KTs[hp][lo:hi,
                                                 t * 128:(t + 1) * 128],
                                    rhs=QTs[hp][lo:hi,
                                                qc * 512:(qc + 1) * 512],
                                    start=True, stop=True)
                        for h2 in range(2):
                            nc.scalar.activation(ATh[h2][:, t, :],
                                                 ps2[h2][:],
                                                 AF.Exp, scale=INV_SQRT_D)
                return ATh

            def att_av(b, stt, hp, ATh, flush_qt=False, tail=False):
                pkv_b = consts[2]
                ctx_p = tc.high_priority(offset=3000)
                ctx_p.__enter__()
                phiQs, vaug, out_sb = stt["phiQs"], stt["vaug"], stt["out_sb"]
                for qt in range(8):
                    for h2 in range(2):
                        h_abs = hp * 2 + h2
                        lo, hi = h2 * 64, (h2 + 1) * 64
                        po = pav.tile([128, DH + 1], F32, tag="av",
                                      name="po")
                        for t in range(8):
                            nc.tensor.matmul(
                                po[:],
                                lhsT=ATh[h2][:, t, qt * 128:(qt + 1) * 128],
                                rhs=vaug[:, t, h_abs, :],
                                start=(t == 0), stop=False,
                                skip_group_check=True)
                        # K=128 with the other head's pkv half zeroed on
                        # host: keeps the PE in 128x128 mode (a 64x128
                        # row-tiled matmul here would force an array
                        # drain on every mode switch).
                        nc.tensor.matmul(
                            po[:],
                            lhsT=phiQs[hp][:, qt * 128:(qt + 1) * 128],
                            rhs=pkv_b[:, h_abs, :],
                            start=False, stop=True, skip_group_check=True)
                        rc = eps_p.tile([128, 1], F32, tag="recip", name="rc")
                        nc.vector.reciprocal(rc[:], po[:, DH:DH + 1])
                        nc.vector.tensor_scalar(
                            out=out_sb[:, qt, h_abs * DH:(h_abs + 1) * DH],
                            in0=po[:, 0:DH], scalar1=rc[:],
                            scalar2=None, op0=ALU.mult)
                    if flush_qt and qt % 2 == 1:
                        pool_eng.dma_start(
                            out=outp[b, :, qt - 1:qt + 1, :],
                            in_=out_sb[:, qt - 1:qt + 1, :])
                ctx_p.__exit__(None, None, None)

            def flush(b, stt):
                for qt in range(8):
                    pool_eng.dma_start(
                        out=outp[b * S + qt * 128: b * S + (qt + 1) * 128, :],
                        in_=stt["out_sb"][:, qt, :])

            # software-pipelined emission: batch 1's preamble is split
            # into chunks interleaved between batch 0's attention blocks so
            # PE/DVE fill the ACT-bound steady state without starving it.
            consts = None
            wres = None
            for _rep in range(reps):
                st0, st1 = new_state(0), new_state(1)
                st0["hT"] = htp.tile([128, 6, S], F16, tag="hT", name="hT_0")
                if wres is None:
                    emit_warmup()
                    wres = alloc_weights()
                    emit_weight_cols("wq", 0, 128)
                    emit_weight_cols("wk", 0, 128)
                    consts = emit_consts_biases()
                    emit_hT(0, st0["hT"])
                    emit_weight_cols("wq", 128, 384)
                    emit_weight_cols("wk", 128, 384)
                    emit_weight_cols("wq", 384, 768)
                    emit_weight_cols("wk", 384, 768)
                    emit_weight_cols("wv", 0, 384)
                    emit_weight_cols("wv", 384, 768)
                    consts = consts + emit_consts_rest()
                else:
                    emit_hT(0, st0["hT"])
                bqt, bkt, pkv_b, bvt = consts

                def qk_pair(b, stt, hp):
                    emit_proj_hp(b, stt, "wq", stt["QTs"], bqt, "wqp",
                                 True, hp)
                    emit_proj_hp(b, stt, "wk", stt["KTs"], bkt, "wkp",
                                 False, hp)

                qk_pair(0, st0, 0)
                a0 = att_qk(0, st0, 0)
                qk_pair(0, st0, 1)
                emit_v(0, st0, 0)
                a1 = att_qk(0, st0, 1)
                qk_pair(0, st0, 2)
                emit_v(0, st0, 1)
                att_av(0, st0, 0, a0)
                a2 = att_qk(0, st0, 2)
                att_av(0, st0, 1, a1)
                qk_pair(0, st0, 3)
                a3 = att_qk(0, st0, 3)
                att_av(0, st0, 2, a2)

                st1["hT"] = htp.tile([128, 6, S], F16, tag="hT", name="hT_1")
                emit_hT(1, st1["hT"])
                qk_pair(0, st0, 4)
                a4 = att_qk(0, st0, 4)
                att_av(0, st0, 3, a3)
                qk_pair(0, st0, 5)
                a5 = att_qk(0, st0, 5)
                att_av(0, st0, 4, a4)
                qk_pair(1, st1, 0)
                att_av(0, st0, 5, a5, flush_qt=True)

                b0 = att_qk(1, st1, 0)
                qk_pair(1, st1, 1)
                emit_v(1, st1, 0)
                b1 = att_qk(1, st1, 1)
                qk_pair(1, st1, 2)
                emit_v(1, st1, 1)
                att_av(1, st1, 0, b0)
                b2 = att_qk(1, st1, 2)
                att_av(1, st1, 1, b1)
                qk_pair(1, st1, 3)
                b3 = att_qk(1, st1, 3)
                att_av(1, st1, 2, b2)
                qk_pair(1, st1, 4)
                b4 = att_qk(1, st1, 4)
                att_av(1, st1, 3, b3)
                qk_pair(1, st1, 5)
                b5 = att_qk(1, st1, 5)
                att_av(1, st1, 4, b4)
                att_av(1, st1, 5, b5, flush_qt=True)
    if split_waits:
        _split_multi_waits(nc)
    return nc


_CACHE = {}


def _prep_in_maps(hidden_states, Wq, bq, Wk, bk, Wv, bv, phi_k, phi_kv):
    f16 = np.float16
    hidden = np.ascontiguousarray(
        np.asarray(hidden_states, np.float32).reshape(B, S, HID)).astype(f16)

    def wt_layout(W):
        # W [o, i] -> W^T in SBUF layout [p_i, i_tile, o]
        return np.ascontiguousarray(
            np.transpose(np.asarray(W, np.float32).reshape(HID, 6, 128),
                         (2, 1, 0)).astype(f16))

    wqt, wkt, wvt = wt_layout(Wq), wt_layout(Wk), wt_layout(Wv)
    bq_p = np.ascontiguousarray(np.asarray(bq, np.float32).reshape(6, 128).T)
    bk_p = np.ascontiguousarray(np.asarray(bk, np.float32).reshape(6, 128).T)
    bv_b = np.ascontiguousarray(
        np.broadcast_to(np.asarray(bv, np.float32), (128, HID)))
    pk = np.abs(np.asarray(phi_k, np.float32).reshape(H, DH, 1))
    pkv = np.asarray(phi_kv, np.float32).reshape(H, DH, DH)
    aug = np.concatenate([pkv, pk], axis=-1)          # [H, DH, 65]
    aug = np.transpose(aug, (1, 0, 2))                # [DH, H, 65]
    # [128, H, 65]: head h's pkv lives on the 64-partition half that
    # matches its phiQ rows (h even -> 0:64, h odd -> 64:128); the other
    # half is zero so the phi matmul can run K=128 (no PE mode switch).
    augz = np.zeros((128, H, DH + 1), np.float32)
    for h in range(H):
        if h % 2 == 0:
            augz[0:64, h, :] = aug[:, h, :]
        else:
            augz[64:128, h, :] = aug[:, h, :]
    aug = np.ascontiguousarray(augz)
    in_maps = []
    for c in range(NCORES):
        in_maps.append({
            "hidden": np.ascontiguousarray(
                hidden[c * BLOC:(c + 1) * BLOC].reshape(SLOC, HID)),
            "wqt": wqt, "wkt": wkt, "wvt": wvt,
            "bq_p": bq_p, "bk_p": bk_p, "bv_b": bv_b,
            "phikv_aug": aug,
        })
    return in_maps


def kernel(hidden_states, Wq, bq, Wk, bk, Wv, bv, phi_k, phi_kv):
    if "nc" not in _CACHE:
        _CACHE["nc"] = build_nc()
    nc = _CACHE["nc"]
    in_maps = _prep_in_maps(hidden_states, Wq, bq, Wk, bk, Wv, bv,
                            phi_k, phi_kv)
    res = run_bass_kernel_spmd(nc, in_maps, list(range(NCORES)), trace=False)
    # device layout is [b, p, qt, c]; row (qt*128+p) order is restored here
    outs = []
    for c in range(NCORES):
        o = np.asarray(res.results[c]["out"])          # [BLOC, 128, 8, HID]
        outs.append(o.transpose(0, 2, 1, 3).reshape(BLOC, S, HID))
    out = np.concatenate(outs, axis=0)
    return np.ascontiguousarray(out.reshape(B, S, HID).astype(np.float32))

